# revision 1
# baseline (speedup 1.0000x reference)
"""Trainium2 Bass kernel: multi-head self-attention with RoPE, causal mask.

Reference semantics (B=2, S=2048, D=1024, H=16, DK=64):
    q = rope(x @ Wq.T), k = rope(x @ Wk.T), v = x @ Wv.T   (per-head views)
    out = softmax(causal(q k^T / 8)) v ;  y = out @ Wo.T

Sharding over 8 cores: 2-way batch x 4-way heads (4 heads/core).
Each core computes a partial y [S, D] (its heads' contribution); host sums
the 4 partials per batch.

On-device layout strategy (per core):
  - host passes xT = x[b].T  [1024, 2048] so the d-contraction is on partitions
  - Q/K projections produce QT/KT [e, s]; head dims are even/odd-interleave
    permuted on the host so RoPE becomes a 32-partition block-swap + two
    multiplies (cos/sin tables with signs baked in)
  - scores are computed TRANSPOSED (k on partitions, q on free) so softmax'd
    probs feed the PV matmul directly as the moving operand
  - no max-subtraction in softmax (scores ~ N(0,1), exp is safe); the
    denominator comes from a ones-column appended to V; normalization is a
    reciprocal + PE-broadcast + multiply at PV-evacuation time
  - all matmul operands are fp32r (11-bit mantissa, 1 PE cycle/row)
"""

import sys

sys.path.insert(0, "/opt/trn_rl_repo")

import numpy as np
import ml_dtypes


S = 2048
D = 1024
NH = 16
DK = 64
HL = 4          # heads per core
EL = HL * DK    # 256 local e-dims
N_CORES = 8
THETA = 10000.0

_compiled = None


def _round_fp32r(x):
    # fp32r matmul operands must be pre-rounded to 11 mantissa bits (RNE)
    xi = np.ascontiguousarray(x, dtype=np.float32).view(np.uint32).astype(np.uint64)
    bias = ((xi >> 12) & 1) + (1 << 11) - 1
    return ((xi + bias) >> 12 << 12).astype(np.uint32).view(np.float32)


def _build():
    import concourse.bacc as bacc
    import concourse.tile as tile
    from concourse import mybir
    from concourse.alu_op_type import AluOpType

    dt = mybir.dt
    f32, f32r = dt.float32, dt.float32r

    nc = bacc.Bacc("TRN2", target_bir_lowering=False, debug=False,
                   num_devices=N_CORES)

    xT = nc.dram_tensor("xT", [D, S], dt.bfloat16, kind="ExternalInput").ap()
    wq = nc.dram_tensor("wq", [D, EL], dt.bfloat16, kind="ExternalInput").ap()
    wq2 = nc.dram_tensor("wq2", [D, EL], dt.bfloat16, kind="ExternalInput").ap()
    wk = nc.dram_tensor("wk", [D, EL], dt.bfloat16, kind="ExternalInput").ap()
    wk2 = nc.dram_tensor("wk2", [D, EL], dt.bfloat16, kind="ExternalInput").ap()
    wv = nc.dram_tensor("wv", [D, EL], dt.bfloat16, kind="ExternalInput").ap()
    wo = nc.dram_tensor("wo", [EL, D], dt.bfloat16, kind="ExternalInput").ap()
    cosT = nc.dram_tensor("cosT", [128, S], f32, kind="ExternalInput").ap()
    sinT = nc.dram_tensor("sinT", [128, S], f32, kind="ExternalInput").ap()
    ident = nc.dram_tensor("ident", [128, 128], f32r, kind="ExternalInput").ap()
    y = nc.dram_tensor("y", [S, D], f32, kind="ExternalOutput").ap()

    with tile.TileContext(nc) as tc:
        with tc.tile_pool(name="persist", bufs=1) as pp:
            # persistent SBUF tiles
            qt = [pp.tile([128, S], dt.bfloat16, tag=f"qt{c}", name=f"qt{c}") for c in range(2)]
            ktz = [[pp.tile([128, S], dt.bfloat16, tag=f"ktz{c}{par}", name=f"ktz{c}{par}")
                   for par in range(2)] for c in range(2)]
            vh = [pp.tile([128, 16 * 65], dt.bfloat16, tag=f"v{h}", name=f"v{h}") for h in range(HL)]
            cos_sb = pp.tile([128, S], f32, tag="cos", name="cos")
            sin_sb = pp.tile([128, S], f32, tag="sin", name="sin")
            id_sb = pp.tile([128, 128], f32r, tag="ident", name="ident")


            # ======== stage A: projections + RoPE + V transpose ========
            # order: V first, then K, then Q — so the attention stream can
            # begin as soon as K and Q chunk0 are rotated, keeping the PE
            # dense across the stage transition (HAM stays warm)
            with tc.tile_pool(name="stagea", bufs=1) as sa, \
                 tc.tile_pool(name="qap", bufs=2) as qap, \
                 tc.tile_pool(name="vtp", bufs=2) as vtp, \
                 tc.tile_pool(name="pa", bufs=4, space="PSUM") as pap, \
                 tc.tile_pool(name="tr", bufs=2, space="PSUM") as trp:

                xts = [sa.tile([128, S], dt.bfloat16, tag=f"xt{dc}", name=f"xt{dc}") for dc in range(8)]
                def load_xt_slices(sg, split=1):
                    for dc in range(8):
                        eng = nc.sync if dc % 2 == 0 else nc.scalar
                        p = 128 // split
                        for j in range(split):
                            eng.dma_start(
                                xts[dc][p * j:p * (j + 1), 512 * sg:512 * (sg + 1)],
                                xT[128 * dc + p * j:128 * dc + p * (j + 1),
                                   512 * sg:512 * (sg + 1)])

                # warm up the PE clock-gate while input DMAs land: dummy
                # matmuls on the (early, tiny) identity tile keep HAM at 8/8
                # so the first real matmuls run at full clock
                nc.scalar.dma_start(id_sb[:], ident)
                idb = id_sb[:].bitcast(dt.bfloat16)   # values irrelevant
                wp = trp.tile([128, 512], f32, tag="warm", name="warm")
                for _ in range(50):
                    nc.tensor.matmul(wp[:, 0:256], idb[:, 0:128], idb[:],
                                     start=True, stop=True)

                # --- V projection (VT then PE-transpose into [s, dv]) ---
                wvs = [sa.tile([128, EL], dt.bfloat16, tag=f"w{dc}", name=f"w{dc}", bufs=3)
                       for dc in range(8)]
                for dc in range(8):
                    nc.sync.dma_start(wvs[dc][:], wv[128 * dc:128 * (dc + 1), :])
                load_xt_slices(0, split=2)
                load_xt_slices(1)
                for half in range(2):
                    sl = slice(1024 * half, 1024 * (half + 1))
                    nc.scalar.dma_start(cos_sb[:, sl], cosT[:, sl])
                    nc.scalar.dma_start(sin_sb[:, sl], sinT[:, sl])
                load_xt_slices(2)
                load_xt_slices(3)

                ones16 = sa.tile([128, 16], f32, tag="ones16", name="ones16")
                nc.vector.memset(ones16[:], 1.0)
                for c in range(2):
                    nc.vector.memset(ktz[c][0][64:128, :], 0.0)
                    nc.vector.memset(ktz[c][1][0:64, :], 0.0)
                for sg in range(4):
                    for ec in range(2):
                        ps = pap.tile([128, 512], f32, tag="pa", name="pa")
                        for dc in range(8):
                            nc.tensor.matmul(
                                ps[:],
                                wvs[dc][:, 128 * ec:128 * (ec + 1)],
                                xts[dc][:, 512 * sg:512 * (sg + 1)],
                                start=(dc == 0), stop=(dc == 7))
                        vt = vtp.tile([128, 512], f32r, tag="vt", name="vt")
                        nc.vector.tensor_copy(vt[:], ps[:])
                        tr = trp.tile([128, 512], f32r, tag="tr", name="tr")
                        for i in range(4):
                            nc.tensor.transpose(
                                tr[:, 128 * i:128 * (i + 1)],
                                vt[:, 128 * i:128 * (i + 1)], id_sb[:])
                        for i in range(4):
                            sc = 4 * sg + i
                            for hh in range(2):
                                h = 2 * ec + hh
                                nc.scalar.copy(
                                    vh[h][:, 65 * sc:65 * sc + 64],
                                    tr[:, 128 * i + 64 * hh:128 * i + 64 * hh + 64])
                for h in range(HL):
                    ones_col = vh[h][:].rearrange(
                        "p (s c) -> p s c", c=65)[:, :, 64:65]
                    nc.vector.tensor_copy(ones_col, ones16[:].rearrange("p (s c) -> p s c", c=1))

                # --- K then Q projections with RoPE ---
                # QA = W x, QAs = W_swapped x (second matmul chain); RoPE is
                # then qa*cos + qas*sin with the multiplies fused onto the
                # PSUM evacuation (no cross-partition swap on the data path)
                for t_i, (w_dram, w2_dram, out_t) in enumerate(
                        [(wk, wk2, None), (wq, wq2, qt)]):
                    ws = [sa.tile([128, EL], dt.bfloat16, tag=f"w{dc}", name=f"w{dc}", bufs=3)
                          for dc in range(8)]
                    for dc in range(8):
                        nc.sync.dma_start(
                            ws[dc][:], w_dram[128 * dc:128 * (dc + 1), :])
                    if t_i == 1:
                        w2s = [sa.tile([128, EL], dt.bfloat16, tag=f"w2{dc}", name=f"w2{dc}", bufs=2)
                               for dc in range(8)]
                        for dc in range(8):
                            nc.scalar.dma_start(
                                w2s[dc][:], w2_dram[128 * dc:128 * (dc + 1), :])
                    for ec in range(2):
                        qa = qap.tile([128, S], f32, tag="qa", name="qa")
                        qas = qap.tile([128, S], f32, tag="qas", name="qas")
                        if t_i == 0:
                            # K: single projection; rotate-half partner via
                            # SBUF-to-SBUF DMA block swap (latency hidden
                            # behind the whole Q phase), then in-place muls
                            for sg in range(4):
                                ps = pap.tile([128, 512], f32, tag="pa", name="pa")
                                for dc in range(8):
                                    nc.tensor.matmul(
                                        ps[:],
                                        ws[dc][:, 128 * ec:128 * (ec + 1)],
                                        xts[dc][:, 512 * sg:512 * (sg + 1)],
                                        start=(dc == 0), stop=(dc == 7))
                                nc.vector.tensor_copy(
                                    qa[:, 512 * sg:512 * (sg + 1)], ps[:])
                            for blk in range(2):
                                b0 = 64 * blk
                                nc.scalar.dma_start(
                                    qas[b0:b0 + 32, :], qa[b0 + 32:b0 + 64, :])
                                nc.scalar.dma_start(
                                    qas[b0 + 32:b0 + 64, :], qa[b0:b0 + 32, :])
                            nc.vector.tensor_mul(qa[:], qa[:], cos_sb[:])
                            nc.vector.tensor_mul(qas[:], qas[:], sin_sb[:])
                        else:
                            for sg in range(4):
                                ps = pap.tile([128, 512], f32, tag="pa", name="pa")
                                for dc in range(8):
                                    nc.tensor.matmul(
                                        ps[:],
                                        ws[dc][:, 128 * ec:128 * (ec + 1)],
                                        xts[dc][:, 512 * sg:512 * (sg + 1)],
                                        start=(dc == 0), stop=(dc == 7))
                                ps2 = pap.tile([128, 512], f32, tag="pa", name="pa2")
                                for dc in range(8):
                                    nc.tensor.matmul(
                                        ps2[:],
                                        w2s[dc][:, 128 * ec:128 * (ec + 1)],
                                        xts[dc][:, 512 * sg:512 * (sg + 1)],
                                        start=(dc == 0), stop=(dc == 7))
                                sl = slice(512 * sg, 512 * (sg + 1))
                                nc.vector.tensor_mul(qa[:, sl], ps[:], cos_sb[:, sl])
                                nc.vector.tensor_mul(qas[:, sl], ps2[:], sin_sb[:, sl])
                        if t_i == 0:
                            # K: write each head-half into its zero-padded
                            # stationary tile (other half stays zero) so score
                            # matmuls run with full K=128 row activity
                            nc.vector.tensor_add(
                                ktz[ec][0][0:64, :], qa[0:64, :], qas[0:64, :])
                            nc.vector.tensor_add(
                                ktz[ec][1][64:128, :], qa[64:128, :], qas[64:128, :])
                        else:
                            nc.vector.tensor_add(out_t[ec][:], qa[:], qas[:])

            # ======== stage B: causal attention + output projection ========
            with tc.tile_pool(name="pb", bufs=1) as pb, \
                 tc.tile_pool(name="ptp", bufs=10) as ptp, \
                 tc.tile_pool(name="nrm", bufs=5) as nrmp, \
                 tc.tile_pool(name="ysb", bufs=2) as ysbp, \
                 tc.tile_pool(name="ps_s", bufs=2, space="PSUM") as ps_s, \
                 tc.tile_pool(name="ps_pv", bufs=4, space="PSUM") as ps_pv, \
                 tc.tile_pool(name="ps_bc", bufs=1, space="PSUM") as ps_bc, \
                 tc.tile_pool(name="ps_y", bufs=1, space="PSUM") as ps_y:

                aot = [pb.tile([128, S], dt.bfloat16, tag=f"aot{c}", name=f"aot{c}") for c in range(2)]
                wo_sb = [pb.tile([128, D], dt.bfloat16, tag=f"wo{c}", name=f"wo{c}") for c in range(2)]
                ones_sb = pb.tile([65, 64], f32r, tag="ones", name="ones")
                onesf = pb.tile([65, 64], f32, tag="onesf", name="onesf")
                nc.vector.memset(onesf[64:65, :], 1.0)
                nc.vector.tensor_copy(ones_sb[64:65, :], onesf[64:65, :])
                for c in range(2):
                    for half in range(2):
                        sl = slice(512 * half, 512 * (half + 1))
                        nc.scalar.dma_start(wo_sb[c][:, sl],
                                            wo[128 * c:128 * (c + 1), sl])

                def out_proj_chunk(sc):
                    ysb = ysbp.tile([128, D], f32, tag="ysb", name="ysb")
                    for eg in range(2):
                        yp = ps_y.tile([128, 512], f32, tag="yp", name="yp")
                        for c2 in range(2):
                            nc.tensor.matmul(
                                yp[:],
                                aot[c2][:, 128 * sc:128 * (sc + 1)],
                                wo_sb[c2][:, 512 * eg:512 * (eg + 1)],
                                start=(c2 == 0), stop=(c2 == 1))
                        nc.vector.tensor_copy(
                            ysb[:, 512 * eg:512 * (eg + 1)], yp[:])
                    for half in range(2):
                        sl = slice(512 * half, 512 * (half + 1))
                        nc.sync.dma_start(
                            y[128 * sc:128 * (sc + 1), sl], ysb[:, sl])

                def out_proj(qg):
                    for i in range(4):
                        out_proj_chunk(4 * qg + i)

                pending = []   # deferred normalize closures (PE work off pair boundary)

                def emit_pending_one():
                    if pending:
                        pending.pop(0)()

                for qg in range(4):
                    n_kc = 4 * qg + 4
                    # interleave deferred work (prev normalize + prev out-proj)
                    # through this q-group's attention stream; full-array
                    # matmuls double as HAM heaters
                    # slot plan: first flush prev q-group's 4 deferred
                    # normalizes, then its 4 out-proj chunks (which read the
                    # aot slices those normalizes write)
                    heat_at = {}
                    norm_at = {}
                    if qg >= 1:
                        k1 = max(1, n_kc // 4)
                        k2 = max(2, n_kc // 2)
                        k3 = max(3, (3 * n_kc) // 4)
                        norm_at = {(0, k1): 4}    # all 4 normalizes, early in hp0
                        heat_at = {(0, k2): 4 * (qg - 1) + 0,
                                   (0, k3): 4 * (qg - 1) + 1,
                                   (1, k1): 4 * (qg - 1) + 2,
                                   (1, k2): 4 * (qg - 1) + 3}
                    for hp in range(2):           # head pairs (0,1), (2,3)
                        ppv = {}
                        for hh in range(2):
                            h = 2 * hp + hh
                            ppv[h] = ps_pv.tile([65, 512], f32, tag="ppv", name="ppv")
                        # software-pipelined: PV lags scores by SKEW so the
                        # exp+mask chain never head-of-line blocks the PE
                        SKEW = 2
                        ptq = {}
                        for kc in range(n_kc + SKEW):
                            for _ in range(norm_at.get((hp, kc), 0)):
                                emit_pending_one()
                            if (hp, kc) in heat_at:
                                out_proj_chunk(heat_at[(hp, kc)])
                            if kc < n_kc:
                                # diagonal tiles only need q >= k: narrow to
                                # the valid q-slice (saves PE+ACT on the
                                # mostly-masked tiles)
                                r = kc - 4 * qg
                                q0 = 128 * r if r > 0 else 0
                                qsl = slice(512 * qg + q0, 512 * (qg + 1))
                                for hh in range(2):
                                    h = 2 * hp + hh
                                    c2, off = hp, 64 * hh
                                    ps = ps_s.tile([128, 512], f32, tag="ps", name="ps")
                                    nc.tensor.matmul(
                                        ps[:, q0:512],
                                        ktz[c2][hh][:, 128 * kc:128 * (kc + 1)],
                                        qt[c2][:, qsl],
                                        start=True, stop=True)
                                    pt = ptp.tile([128, 512], dt.bfloat16, tag="pt", name="pt")
                                    nc.scalar.activation(
                                        pt[:, q0:512], ps[:, q0:512],
                                        mybir.ActivationFunctionType.Exp,
                                        scale=0.125)
                                    if r >= 0:
                                        nc.gpsimd.affine_select(
                                            pt[:, q0:512], pt[:, q0:512],
                                            pattern=[[1, 512 - q0]],
                                            compare_op=AluOpType.is_ge, fill=0.0,
                                            base=512 * qg + q0 - 128 * kc,
                                            channel_multiplier=-1)
                                    ptq[(h, kc)] = (pt, q0)
                            kcp = kc - SKEW
                            if kcp >= 0:
                                for hh in range(2):
                                    h = 2 * hp + hh
                                    ptv, q0v = ptq.pop((h, kcp))
                                    nc.tensor.matmul(
                                        ppv[h][:, q0v:512],
                                        vh[h][:, 65 * kcp:65 * kcp + 65],
                                        ptv[:, q0v:512],
                                        start=(kcp == 0), stop=(kcp == n_kc - 1))
                        # release ppv quickly: DVE-copy denom + attn-out to
                        # SBUF; defer the PE bcast + normalize multiply
                        for hh in range(2):
                            h = 2 * hp + hh
                            c2, off = hp, 64 * hh
                            dao = nrmp.tile([65, 512], f32, tag="dao", name="dao")
                            nc.vector.tensor_copy(dao[:], ppv[h][:])
                            ao = dao[0:64, :]
                            rec = nrmp.tile([65, 512], f32, tag="rec", name="rec")
                            nc.vector.reciprocal(rec[64:65, :], dao[64:65, :])
                            recr = nrmp.tile([65, 512], f32r, tag="recr", name="recr")
                            nc.vector.tensor_copy(recr[64:65, :], rec[64:65, :])

                            def mk_norm(qg=qg, c2=c2, off=off, recr=recr, ao=ao):
                                def emit():
                                    bc = ps_bc.tile([64, 512], f32, tag="bc", name="bc")
                                    nc.tensor.matmul(bc[:], ones_sb[64:65, :],
                                                     recr[64:65, :],
                                                     start=True, stop=True)
                                    nc.vector.tensor_mul(
                                        aot[c2][off:off + 64, 512 * qg:512 * (qg + 1)],
                                        ao[:], bc[:])
                                return emit
                            pending.append(mk_norm())
                while pending:
                    emit_pending_one()
                out_proj(3)

    nc.compile()
    return nc


def _prep_inputs(x, token_positions, Wq, Wk, Wv, Wo):
    # even/odd interleave permutation within each head (for rotate-half RoPE)
    perm = np.concatenate([np.arange(0, DK, 2), np.arange(1, DK, 2)])

    pos = np.asarray(token_positions).astype(np.float32)
    angles = THETA ** (-np.arange(32, dtype=np.float32) / 32.0)
    ang = pos[:, None] * angles[None, :]          # [S, 32]
    cos32 = np.cos(ang).T.astype(np.float32)      # [32, S]
    sin32 = np.sin(ang).T.astype(np.float32)
    cos128 = np.concatenate([cos32, cos32, cos32, cos32], axis=0)
    sin128 = np.concatenate([-sin32, sin32, -sin32, sin32], axis=0)
    cos128 = np.ascontiguousarray(cos128)
    sin128 = np.ascontiguousarray(sin128)

    identity = _round_fp32r(np.eye(128, dtype=np.float32))

    Wq = np.asarray(Wq, dtype=np.float32)
    Wk = np.asarray(Wk, dtype=np.float32)
    Wv = np.asarray(Wv, dtype=np.float32)
    Wo = np.asarray(Wo, dtype=np.float32)
    x = np.asarray(x, dtype=np.float32)

    in_maps = []
    for c in range(N_CORES):
        b = c // 4
        h0 = (c % 4) * HL
        esl = slice(h0 * DK, (h0 + HL) * DK)
        swap = np.concatenate([np.arange(32, 64), np.arange(0, 32)])
        wq_h = Wq[esl].reshape(HL, DK, D)[:, perm]
        wk_h = Wk[esl].reshape(HL, DK, D)[:, perm]
        wq2_h = wq_h[:, swap].reshape(EL, D)
        wk2_h = wk_h[:, swap].reshape(EL, D)
        wq_h = wq_h.reshape(EL, D)
        wk_h = wk_h.reshape(EL, D)
        wv_h = Wv[esl]
        bf = lambda a: np.ascontiguousarray(a, dtype=np.float32).astype(ml_dtypes.bfloat16)
        in_maps.append({
            "xT": bf(x[b].T),
            "wq": bf(wq_h.T),
            "wq2": bf(wq2_h.T),
            "wk": bf(wk_h.T),
            "wk2": bf(wk2_h.T),
            "wv": bf(wv_h.T),
            "wo": bf(Wo[:, esl].T),
            "cosT": cos128,
            "sinT": sin128,
            "ident": identity,
        })
    return in_maps


def kernel(x, token_positions, Wq, Wk, Wv, Wo, _trace=False):
    from concourse.bass_utils import run_bass_kernel_spmd

    global _compiled
    if _compiled is None:
        _compiled = _build()
    in_maps = _prep_inputs(x, token_positions, Wq, Wk, Wv, Wo)
    res = run_bass_kernel_spmd(_compiled, in_maps, list(range(N_CORES)),
                               trace=_trace)
    parts = [res.results[c]["y"].astype(np.float64) for c in range(N_CORES)]
    out = np.empty((2, S, D), dtype=np.float32)
    out[0] = (parts[0] + parts[1] + parts[2] + parts[3]).astype(np.float32)
    out[1] = (parts[4] + parts[5] + parts[6] + parts[7]).astype(np.float32)
    if _trace:
        return out, res
    return out



# revision 4
# speedup vs baseline: 1.1030x; 1.1030x over previous
"""Trainium2 Bass kernel: multi-head self-attention with RoPE, causal mask.

Reference semantics (B=2, S=2048, D=1024, H=16, DK=64):
    q = rope(x @ Wq.T), k = rope(x @ Wk.T), v = x @ Wv.T   (per-head views)
    out = softmax(causal(q k^T / 8)) v ;  y = out @ Wo.T

Sharding over 8 cores: 2-way batch x 4-way heads (4 heads/core).
Each core computes a partial y [S, D] (its heads' contribution); host sums
the 4 partials per batch.

On-device layout strategy (per core):
  - host passes xT = x[b].T  [1024, 2048] so the d-contraction is on partitions
  - Q/K projections produce QT/KT [e, s]; head dims are even/odd-interleave
    permuted on the host so RoPE becomes a 32-partition block-swap + two
    multiplies (cos/sin tables with signs baked in)
  - Q and K are each projected ONCE; the rotate-half partner comes from a
    per-512-chunk SBUF-to-SBUF DMA block swap (issued on the idle gpsimd
    queue), with the rope multiplies chunked so they overlap the remaining
    projection matmuls on the PE
  - PSUM evacuations of the Q/K projections ride the ACT (scalar) engine,
    which is otherwise idle in stage A
  - scores are computed TRANSPOSED (k on partitions, q on free) so softmax'd
    probs feed the PV matmul directly as the moving operand
  - the two heads of a pair write their score tiles into ONE two-bank PSUM
    tile, so a single Exp activation instruction covers both (halves the
    ACT-engine instruction count, which binds stage B)
  - no max-subtraction in softmax (scores ~ N(0,1), exp is safe); the
    denominator comes from a ones-column appended to V; normalization is a
    fast approximate reciprocal + gpsimd partition-broadcast + multiply
  - all matmul operands are fp32r/bf16 (1 PE cycle/row)
"""

import sys

sys.path.insert(0, "/opt/trn_rl_repo")

import numpy as np
import ml_dtypes


S = 2048
D = 1024
NH = 16
DK = 64
HL = 4          # heads per core
EL = HL * DK    # 256 local e-dims
N_CORES = 8
THETA = 10000.0

_compiled = None


def _round_fp32r(x):
    # fp32r matmul operands must be pre-rounded to 11 mantissa bits (RNE)
    xi = np.ascontiguousarray(x, dtype=np.float32).view(np.uint32).astype(np.uint64)
    bias = ((xi >> 12) & 1) + (1 << 11) - 1
    return ((xi + bias) >> 12 << 12).astype(np.uint32).view(np.float32)


def _build():
    import concourse.bacc as bacc
    import concourse.tile as tile
    from concourse import mybir
    from concourse import library_config
    from concourse.alu_op_type import AluOpType

    dt = mybir.dt
    f32, f32r = dt.float32, dt.float32r

    nc = bacc.Bacc("TRN2", target_bir_lowering=False, debug=False,
                   num_devices=N_CORES)

    xT = nc.dram_tensor("xT", [D, S], dt.bfloat16, kind="ExternalInput").ap()
    wq = nc.dram_tensor("wq", [D, EL], dt.bfloat16, kind="ExternalInput").ap()
    wk = nc.dram_tensor("wk", [D, EL], dt.bfloat16, kind="ExternalInput").ap()
    wv = nc.dram_tensor("wv", [D, EL], dt.bfloat16, kind="ExternalInput").ap()
    wo = nc.dram_tensor("wo", [EL, D], dt.bfloat16, kind="ExternalInput").ap()
    cosT = nc.dram_tensor("cosT", [128, S], f32, kind="ExternalInput").ap()
    sinT = nc.dram_tensor("sinT", [128, S], f32, kind="ExternalInput").ap()
    ident = nc.dram_tensor("ident", [128, 128], f32r, kind="ExternalInput").ap()
    y = nc.dram_tensor("y", [S, D], dt.float16, kind="ExternalOutput").ap()

    with tile.TileContext(nc) as tc:
        with tc.tile_pool(name="persist", bufs=1) as pp:
            # persistent SBUF tiles
            qt = [pp.tile([128, S], dt.bfloat16, tag=f"qt{c}", name=f"qt{c}") for c in range(2)]
            ktz = [[pp.tile([128, S], dt.bfloat16, tag=f"ktz{c}{par}", name=f"ktz{c}{par}")
                   for par in range(2)] for c in range(2)]
            vh = [pp.tile([128, 16 * 65], dt.bfloat16, tag=f"v{h}", name=f"v{h}") for h in range(HL)]
            cos_sb = pp.tile([128, S], f32, tag="cos", name="cos")
            sin_sb = pp.tile([128, S], f32, tag="sin", name="sin")
            id_sb = pp.tile([128, 128], f32r, tag="ident", name="ident")

            # gpsimd library with partition_broadcast (used for softmax
            # denominator broadcast); load while everything is idle
            nc.gpsimd.load_library(library_config.attn)

            # ======== stage A: projections + RoPE + V transpose ========
            # order: V first, then K, then Q — so the attention stream can
            # begin as soon as K and Q are rotated, keeping the PE dense
            # across the stage transition (HAM stays warm)
            with tc.tile_pool(name="stagea", bufs=1) as sa, \
                 tc.tile_pool(name="qap", bufs=2) as qap, \
                 tc.tile_pool(name="rop", bufs=3) as rop, \
                 tc.tile_pool(name="vtp", bufs=2) as vtp, \
                 tc.tile_pool(name="pa", bufs=4, space="PSUM") as pap, \
                 tc.tile_pool(name="tr", bufs=2, space="PSUM") as trp:

                xts = [sa.tile([128, S], dt.bfloat16, tag=f"xt{dc}", name=f"xt{dc}") for dc in range(8)]
                def load_xt_slices(sg, split=1):
                    for dc in range(8):
                        eng = nc.sync if dc % 2 == 0 else nc.scalar
                        p = 128 // split
                        for j in range(split):
                            eng.dma_start(
                                xts[dc][p * j:p * (j + 1), 512 * sg:512 * (sg + 1)],
                                xT[128 * dc + p * j:128 * dc + p * (j + 1),
                                   512 * sg:512 * (sg + 1)])

                # warm up the PE clock-gate while input DMAs land: dummy
                # matmuls on the (early, tiny) identity tile keep HAM at 8/8
                # so the first real matmuls run at full clock
                nc.scalar.dma_start(id_sb[:], ident)
                idb = id_sb[:].bitcast(dt.bfloat16)   # values irrelevant
                wp = trp.tile([128, 512], f32, tag="warm", name="warm")
                for _ in range(50):
                    nc.tensor.matmul(wp[:, 0:256], idb[:, 0:128], idb[:],
                                     start=True, stop=True)

                # --- V projection (VT then PE-transpose into [s, dv]) ---
                wvs = [sa.tile([128, EL], dt.bfloat16, tag=f"w{dc}", name=f"w{dc}", bufs=3)
                       for dc in range(8)]
                for dc in range(8):
                    nc.sync.dma_start(wvs[dc][:], wv[128 * dc:128 * (dc + 1), :])
                load_xt_slices(0, split=2)
                load_xt_slices(1)
                for half in range(2):
                    sl = slice(1024 * half, 1024 * (half + 1))
                    nc.scalar.dma_start(cos_sb[:, sl], cosT[:, sl])
                    nc.scalar.dma_start(sin_sb[:, sl], sinT[:, sl])
                load_xt_slices(2)
                load_xt_slices(3)

                ones16 = sa.tile([128, 16], f32, tag="ones16", name="ones16")
                nc.vector.memset(ones16[:], 1.0)
                for c in range(2):
                    nc.gpsimd.memset(ktz[c][0][64:128, :], 0.0)
                    nc.gpsimd.memset(ktz[c][1][0:64, :], 0.0)
                for sg in range(4):
                    for ec in range(2):
                        ps = pap.tile([128, 512], f32, tag="pa", name="pa")
                        for dc in range(8):
                            nc.tensor.matmul(
                                ps[:],
                                wvs[dc][:, 128 * ec:128 * (ec + 1)],
                                xts[dc][:, 512 * sg:512 * (sg + 1)],
                                start=(dc == 0), stop=(dc == 7))
                        vt = vtp.tile([128, 512], f32r, tag="vt", name="vt")
                        nc.vector.tensor_copy(vt[:], ps[:])
                        tr = trp.tile([128, 512], f32r, tag="tr", name="tr")
                        for i in range(4):
                            nc.tensor.transpose(
                                tr[:, 128 * i:128 * (i + 1)],
                                vt[:, 128 * i:128 * (i + 1)], id_sb[:])
                        for i in range(4):
                            sc = 4 * sg + i
                            for hh in range(2):
                                h = 2 * ec + hh
                                nc.scalar.copy(
                                    vh[h][:, 65 * sc:65 * sc + 64],
                                    tr[:, 128 * i + 64 * hh:128 * i + 64 * hh + 64])
                for h in range(HL):
                    ones_col = vh[h][:].rearrange(
                        "p (s c) -> p s c", c=65)[:, :, 64:65]
                    nc.vector.tensor_copy(ones_col, ones16[:].rearrange("p (s c) -> p s c", c=1))

                # --- K then Q projections with RoPE (both single-matmul) ---
                # per 512-col chunk: project -> ACT-evacuate to SBUF ->
                # DMA block-swap (rotate-half partner) -> DVE cos/sin
                # multiplies + add.  The vector/DMA tail of chunk i overlaps
                # the PE chain of chunk i+1.
                for t_i, w_dram in enumerate([wk, wq]):
                    ws = [sa.tile([128, EL], dt.bfloat16, tag=f"w{dc}", name=f"w{dc}", bufs=3)
                          for dc in range(8)]
                    for dc in range(8):
                        nc.sync.dma_start(
                            ws[dc][:], w_dram[128 * dc:128 * (dc + 1), :])
                    for ec in range(2):
                        qa = qap.tile([128, S], f32, tag="qa", name="qa")
                        qas = qap.tile([128, S], f32, tag="qas", name="qas")
                        for sg in range(4):
                            sl = slice(512 * sg, 512 * (sg + 1))
                            ps = pap.tile([128, 512], f32, tag="pa", name="pa")
                            for dc in range(8):
                                nc.tensor.matmul(
                                    ps[:],
                                    ws[dc][:, 128 * ec:128 * (ec + 1)],
                                    xts[dc][:, 512 * sg:512 * (sg + 1)],
                                    start=(dc == 0), stop=(dc == 7))
                            nc.scalar.copy(qa[:, sl], ps[:])
                            # rotate-half partner: swap 32-partition blocks
                            for blk in range(2):
                                b0 = 64 * blk
                                nc.sync.dma_start(
                                    qas[b0:b0 + 32, sl], qa[b0 + 32:b0 + 64, sl])
                                nc.sync.dma_start(
                                    qas[b0 + 32:b0 + 64, sl], qa[b0:b0 + 32, sl])
                            qc = rop.tile([128, 512], f32, tag="qc", name="qc")
                            qs = rop.tile([128, 512], f32, tag="qs", name="qs")
                            nc.vector.tensor_mul(qc[:], qa[:, sl], cos_sb[:, sl])
                            nc.vector.tensor_mul(qs[:], qas[:, sl], sin_sb[:, sl])
                            if t_i == 0:
                                # K: write each head-half into its zero-padded
                                # stationary tile (other half stays zero) so
                                # score matmuls run with full K=128 activity
                                nc.vector.tensor_add(
                                    ktz[ec][0][0:64, sl], qc[0:64, :], qs[0:64, :])
                                nc.vector.tensor_add(
                                    ktz[ec][1][64:128, sl], qc[64:128, :], qs[64:128, :])
                            else:
                                nc.vector.tensor_add(qt[ec][:, sl], qc[:], qs[:])

            # ======== stage B: causal attention + output projection ========
            with tc.tile_pool(name="pb", bufs=1) as pb, \
                 tc.tile_pool(name="ptp", bufs=8) as ptp, \
                 tc.tile_pool(name="nrm", bufs=4) as nrmp, \
                 tc.tile_pool(name="ysb", bufs=2) as ysbp, \
                 tc.tile_pool(name="ps_s", bufs=2, space="PSUM") as ps_s, \
                 tc.tile_pool(name="ps_pv", bufs=2, space="PSUM") as ps_pv, \
                 tc.tile_pool(name="ps_y", bufs=1, space="PSUM") as ps_y, \
                 tc.tile_pool(name="ps_bc", bufs=1, space="PSUM") as ps_bc:

                aot = [pb.tile([128, S], dt.bfloat16, tag=f"aot{c}", name=f"aot{c}") for c in range(2)]
                wo_sb = [pb.tile([128, D], dt.bfloat16, tag=f"wo{c}", name=f"wo{c}") for c in range(2)]
                ones_sb = pb.tile([65, 64], f32r, tag="ones", name="ones")
                onesf = pb.tile([65, 64], f32, tag="onesf", name="onesf")
                nc.vector.memset(onesf[64:65, :], 1.0)
                nc.vector.tensor_copy(ones_sb[64:65, :], onesf[64:65, :])
                for c in range(2):
                    for half in range(2):
                        sl = slice(512 * half, 512 * (half + 1))
                        nc.scalar.dma_start(wo_sb[c][:, sl],
                                            wo[128 * c:128 * (c + 1), sl])

                def out_proj_chunk(sc):
                    ysb = ysbp.tile([128, D], dt.float16, tag="ysb", name="ysb")
                    for eg in range(2):
                        yp = ps_y.tile([128, 512], f32, tag="yp", name="yp")
                        for c2 in range(2):
                            nc.tensor.matmul(
                                yp[:],
                                aot[c2][:, 128 * sc:128 * (sc + 1)],
                                wo_sb[c2][:, 512 * eg:512 * (eg + 1)],
                                start=(c2 == 0), stop=(c2 == 1))
                        nc.vector.tensor_copy(
                            ysb[:, 512 * eg:512 * (eg + 1)], yp[:])
                    for half in range(2):
                        sl = slice(512 * half, 512 * (half + 1))
                        nc.sync.dma_start(
                            y[128 * sc:128 * (sc + 1), sl], ysb[:, sl])

                pending = []   # deferred normalize closures

                def emit_pending_one():
                    if pending:
                        pending.pop(0)()

                SKEW = 3
                for qg in range(4):
                    n_kc = 4 * qg + 4
                    # interleave deferred work (prev normalize + prev
                    # out-proj) through this q-group's attention stream
                    heat_at = {}
                    norm_at = {}
                    if qg >= 1:
                        k1 = max(1, n_kc // 4)
                        k2 = max(2, n_kc // 2)
                        k3 = max(3, (3 * n_kc) // 4)
                        norm_at = {(0, k1): 4}    # all 4 normalizes, early in hp0
                        heat_at = {(0, k2): 4 * (qg - 1) + 0,
                                   (0, k3): 4 * (qg - 1) + 1,
                                   (1, k1): 4 * (qg - 1) + 2,
                                   (1, k2): 4 * (qg - 1) + 3}
                    if qg == 3:
                        # flush this qg's hp0 normalizes during hp1 so only
                        # hp1's pair remains for the tail
                        norm_at[(1, 10)] = 2
                    for hp in range(2):           # head pairs (0,1), (2,3)
                        ppv = {}
                        for hh in range(2):
                            h = 2 * hp + hh
                            ppv[h] = ps_pv.tile([65, 512], f32, tag="ppv", name="ppv")
                        # software-pipelined: PV lags scores by SKEW so the
                        # exp+mask chain never head-of-line blocks the PE
                        ptq = {}
                        for kc in range(n_kc + SKEW):
                            for _ in range(norm_at.get((hp, kc), 0)):
                                emit_pending_one()
                            if (hp, kc) in heat_at:
                                out_proj_chunk(heat_at[(hp, kc)])
                            if kc < n_kc:
                                # diagonal tiles only need q >= k: narrow to
                                # the valid q-slice (saves PE+ACT on the
                                # mostly-masked tiles)
                                r = kc - 4 * qg
                                q0 = 128 * r if r > 0 else 0
                                qsl = slice(512 * qg + q0, 512 * (qg + 1))
                                ps2 = ps_s.tile([128, 1024], f32, tag="ps", name="ps")
                                for hh in range(2):
                                    nc.tensor.matmul(
                                        ps2[:, 512 * hh + q0:512 * (hh + 1)],
                                        ktz[hp][hh][:, 128 * kc:128 * (kc + 1)],
                                        qt[hp][:, qsl],
                                        start=True, stop=True)
                                pt = ptp.tile([128, 1024], dt.bfloat16, tag="pt", name="pt")
                                # one Exp over both heads' tiles (3D AP view)
                                psv = ps2[:].rearrange("p (h q) -> p h q", h=2)[:, :, q0:512]
                                ptv = pt[:].rearrange("p (h q) -> p h q", h=2)[:, :, q0:512]
                                nc.scalar.activation(
                                    ptv, psv,
                                    mybir.ActivationFunctionType.Exp,
                                    scale=0.125)
                                if r >= 0:
                                    for hh in range(2):
                                        nc.gpsimd.affine_select(
                                            pt[:, 512 * hh + q0:512 * (hh + 1)],
                                            pt[:, 512 * hh + q0:512 * (hh + 1)],
                                            pattern=[[1, 512 - q0]],
                                            compare_op=AluOpType.is_ge, fill=0.0,
                                            base=512 * qg + q0 - 128 * kc,
                                            channel_multiplier=-1)
                                ptq[kc] = (pt, q0)
                            kcp = kc - SKEW
                            if kcp >= 0:
                                ptv2, q0v = ptq.pop(kcp)
                                for hh in range(2):
                                    h = 2 * hp + hh
                                    nc.tensor.matmul(
                                        ppv[h][:, q0v:512],
                                        vh[h][:, 65 * kcp:65 * kcp + 65],
                                        ptv2[:, 512 * hh + q0v:512 * (hh + 1)],
                                        start=(kcp == 0), stop=(kcp == n_kc - 1))
                        # evacuate ppv fast: BOTH attn-out+denom copies first
                        # (they gate PSUM reuse for the next head pair), then
                        # the cheap approximate reciprocals
                        daos = []
                        for hh in range(2):
                            h = 2 * hp + hh
                            dao = nrmp.tile([65, 512], f32, tag="dao", name="dao")
                            nc.vector.tensor_copy(dao[:], ppv[h][:])
                            daos.append(dao)
                        for hh in range(2):
                            dao = daos[hh]
                            rec = nrmp.tile([65, 512], f32, tag="rec", name="rec")
                            nc.vector.reciprocal(
                                rec[64:65, :], dao[64:65, :])
                            recr = nrmp.tile([65, 512], f32r, tag="recr", name="recr")
                            nc.vector.tensor_copy(recr[64:65, :], rec[64:65, :])

                            def mk_norm(qg=qg, c2=hp, off=64 * hh, recr=recr, dao=dao):
                                def emit():
                                    bc = ps_bc.tile([64, 512], f32, tag="bc", name="bc")
                                    nc.tensor.matmul(bc[:], ones_sb[64:65, :],
                                                     recr[64:65, :],
                                                     start=True, stop=True)
                                    nc.vector.tensor_mul(
                                        aot[c2][off:off + 64, 512 * qg:512 * (qg + 1)],
                                        dao[0:64, :], bc[:])
                                return emit
                            pending.append(mk_norm())
                # tail: the two remaining hp1 normalizes, then the last four
                # out-projection chunks
                while pending:
                    emit_pending_one()
                for i in range(4):
                    out_proj_chunk(12 + i)

    nc.compile()
    return nc


def _prep_inputs(x, token_positions, Wq, Wk, Wv, Wo):
    # even/odd interleave permutation within each head (for rotate-half RoPE)
    perm = np.concatenate([np.arange(0, DK, 2), np.arange(1, DK, 2)])

    pos = np.asarray(token_positions).astype(np.float32)
    angles = THETA ** (-np.arange(32, dtype=np.float32) / 32.0)
    ang = pos[:, None] * angles[None, :]          # [S, 32]
    cos32 = np.cos(ang).T.astype(np.float32)      # [32, S]
    sin32 = np.sin(ang).T.astype(np.float32)
    cos128 = np.concatenate([cos32, cos32, cos32, cos32], axis=0)
    sin128 = np.concatenate([-sin32, sin32, -sin32, sin32], axis=0)
    cos128 = np.ascontiguousarray(cos128)
    sin128 = np.ascontiguousarray(sin128)

    identity = _round_fp32r(np.eye(128, dtype=np.float32))

    Wq = np.asarray(Wq, dtype=np.float32)
    Wk = np.asarray(Wk, dtype=np.float32)
    Wv = np.asarray(Wv, dtype=np.float32)
    Wo = np.asarray(Wo, dtype=np.float32)
    x = np.asarray(x, dtype=np.float32)

    in_maps = []
    for c in range(N_CORES):
        b = c // 4
        h0 = (c % 4) * HL
        esl = slice(h0 * DK, (h0 + HL) * DK)
        wq_h = Wq[esl].reshape(HL, DK, D)[:, perm].reshape(EL, D)
        wk_h = Wk[esl].reshape(HL, DK, D)[:, perm].reshape(EL, D)
        wv_h = Wv[esl]
        bf = lambda a: np.ascontiguousarray(a, dtype=np.float32).astype(ml_dtypes.bfloat16)
        in_maps.append({
            "xT": bf(x[b].T),
            "wq": bf(wq_h.T),
            "wk": bf(wk_h.T),
            "wv": bf(wv_h.T),
            "wo": bf(Wo[:, esl].T),
            "cosT": cos128,
            "sinT": sin128,
            "ident": identity,
        })
    return in_maps


def kernel(x, token_positions, Wq, Wk, Wv, Wo, _trace=False):
    from concourse.bass_utils import run_bass_kernel_spmd

    global _compiled
    if _compiled is None:
        _compiled = _build()
    in_maps = _prep_inputs(x, token_positions, Wq, Wk, Wv, Wo)
    res = run_bass_kernel_spmd(_compiled, in_maps, list(range(N_CORES)),
                               trace=_trace)
    parts = [res.results[c]["y"].astype(np.float64) for c in range(N_CORES)]
    out = np.empty((2, S, D), dtype=np.float32)
    out[0] = (parts[0] + parts[1] + parts[2] + parts[3]).astype(np.float32)
    out[1] = (parts[4] + parts[5] + parts[6] + parts[7]).astype(np.float32)
    if _trace:
        return out, res
    return out


# revision 9
# speedup vs baseline: 1.3633x; 1.2360x over previous
"""Trainium2 Bass kernel: multi-head self-attention with RoPE, causal mask.

Reference semantics (B=2, S=2048, D=1024, H=16, DK=64):
    q = rope(x @ Wq.T), k = rope(x @ Wk.T), v = x @ Wv.T   (per-head views)
    out = softmax(causal(q k^T / 8)) v ;  y = out @ Wo.T

Sharding over 8 cores: 2-way batch x 4-way heads (4 heads/core).
Each core computes a partial y [S, D] (its heads' contribution); host sums
the 4 partials per batch.

On-device layout strategy (per core):
  - host passes xT = x[b].T  [1024, 2048] so the d-contraction is on partitions
  - Q/K projections produce QT/KT [e, s]; head dims are even/odd-interleave
    permuted on the host so RoPE becomes a 32-partition block-swap + two
    multiplies (cos/sin tables with signs baked in)
  - Q and K are each projected ONCE; the rotate-half partner comes from a
    per-512-chunk SBUF-to-SBUF DMA block swap (issued on the idle gpsimd
    queue), with the rope multiplies chunked so they overlap the remaining
    projection matmuls on the PE
  - PSUM evacuations of the Q/K projections ride the ACT (scalar) engine,
    which is otherwise idle in stage A
  - scores are computed TRANSPOSED (k on partitions, q on free) so softmax'd
    probs feed the PV matmul directly as the moving operand
  - the two heads of a pair write their score tiles into ONE two-bank PSUM
    tile, so a single Exp activation instruction covers both (halves the
    ACT-engine instruction count, which binds stage B)
  - no max-subtraction in softmax (scores ~ N(0,1), exp is safe); the
    denominator comes from a ones-column appended to V; normalization is a
    fast approximate reciprocal + gpsimd partition-broadcast + multiply
  - all matmul operands are fp32r/bf16 (1 PE cycle/row)
"""

import sys

sys.path.insert(0, "/opt/trn_rl_repo")

import numpy as np
import ml_dtypes


S = 2048
D = 1024
NH = 16
DK = 64
HL = 4          # heads per core
EL = HL * DK    # 256 local e-dims
N_CORES = 8
THETA = 10000.0

_compiled = None


def _round_fp32r(x):
    # fp32r matmul operands must be pre-rounded to 11 mantissa bits (RNE)
    xi = np.ascontiguousarray(x, dtype=np.float32).view(np.uint32).astype(np.uint64)
    bias = ((xi >> 12) & 1) + (1 << 11) - 1
    return ((xi + bias) >> 12 << 12).astype(np.uint32).view(np.float32)


def _build():
    import concourse.bacc as bacc
    import concourse.tile as tile
    from concourse import mybir
    from concourse.alu_op_type import AluOpType

    dt = mybir.dt
    f32, f32r = dt.float32, dt.float32r

    nc = bacc.Bacc("TRN2", target_bir_lowering=False, debug=False,
                   num_devices=N_CORES)

    xT = nc.dram_tensor("xT", [D, S], dt.bfloat16, kind="ExternalInput").ap()
    wq = nc.dram_tensor("wq", [D, EL], dt.bfloat16, kind="ExternalInput").ap()
    wk = nc.dram_tensor("wk", [D, EL], dt.bfloat16, kind="ExternalInput").ap()
    wv = nc.dram_tensor("wv", [D, EL], dt.bfloat16, kind="ExternalInput").ap()
    wo = nc.dram_tensor("wo", [EL, D], dt.bfloat16, kind="ExternalInput").ap()
    cosT = nc.dram_tensor("cosT", [128, S], f32, kind="ExternalInput").ap()
    sinT = nc.dram_tensor("sinT", [128, S], f32, kind="ExternalInput").ap()
    ident = nc.dram_tensor("ident", [128, 128], f32r, kind="ExternalInput").ap()
    y = nc.dram_tensor("y", [S, D], dt.float16, kind="ExternalOutput").ap()

    with tile.TileContext(nc) as tc:
        with tc.tile_pool(name="persist", bufs=1) as pp:
            # persistent SBUF tiles
            qt = [pp.tile([128, S], dt.bfloat16, tag=f"qt{c}", name=f"qt{c}") for c in range(2)]
            ktz = [[pp.tile([128, S], dt.bfloat16, tag=f"ktz{c}{par}", name=f"ktz{c}{par}")
                   for par in range(2)] for c in range(2)]
            vh = [pp.tile([128, 16 * 128], dt.bfloat16, tag=f"v{h}", name=f"v{h}") for h in range(HL)]
            cos_sb = pp.tile([128, S], f32, tag="cos", name="cos")
            sin_sb = pp.tile([128, S], f32, tag="sin", name="sin")
            id_sb = pp.tile([128, 128], f32r, tag="ident", name="ident")

            # ======== stage A: projections + RoPE + V transpose ========
            # order: V first, then K, then Q — so the attention stream can
            # begin as soon as K and Q are rotated, keeping the PE dense
            # across the stage transition (HAM stays warm)
            with tc.tile_pool(name="stagea", bufs=1) as sa, \
                 tc.tile_pool(name="qap", bufs=2) as qap, \
                 tc.tile_pool(name="rop", bufs=3) as rop, \
                 tc.tile_pool(name="vtp", bufs=2) as vtp, \
                 tc.tile_pool(name="pa", bufs=4, space="PSUM") as pap, \
                 tc.tile_pool(name="tr", bufs=2, space="PSUM") as trp:

                xts = [sa.tile([128, S], dt.bfloat16, tag=f"xt{dc}", name=f"xt{dc}") for dc in range(8)]
                def load_xt_slices(sg, split=1):
                    for dc in range(8):
                        eng = nc.sync if dc % 2 == 0 else nc.scalar
                        p = 128 // split
                        for j in range(split):
                            eng.dma_start(
                                xts[dc][p * j:p * (j + 1), 512 * sg:512 * (sg + 1)],
                                xT[128 * dc + p * j:128 * dc + p * (j + 1),
                                   512 * sg:512 * (sg + 1)])

                # warm up the PE clock-gate while input DMAs land: dummy
                # matmuls on the (early, tiny) identity tile keep HAM at 8/8
                # so the first real matmuls run at full clock
                nc.scalar.dma_start(id_sb[:], ident)
                idb = id_sb[:].bitcast(dt.bfloat16)   # values irrelevant
                wp = trp.tile([128, 512], f32, tag="warm", name="warm")
                for _ in range(50):
                    nc.tensor.matmul(wp[:, 0:256], idb[:, 0:128], idb[:],
                                     start=True, stop=True)

                # --- V projection (VT then PE-transpose into [s, dv]) ---
                wvs = [sa.tile([128, EL], dt.bfloat16, tag=f"w{dc}", name=f"w{dc}", bufs=3)
                       for dc in range(8)]
                for dc in range(8):
                    nc.sync.dma_start(wvs[dc][:], wv[128 * dc:128 * (dc + 1), :])
                load_xt_slices(0, split=2)
                load_xt_slices(1)
                for half in range(2):
                    sl = slice(1024 * half, 1024 * (half + 1))
                    nc.scalar.dma_start(cos_sb[:, sl], cosT[:, sl])
                    nc.scalar.dma_start(sin_sb[:, sl], sinT[:, sl])
                load_xt_slices(2)
                load_xt_slices(3)

                ones16 = sa.tile([128, 16], f32, tag="ones16", name="ones16")
                nc.vector.memset(ones16[:], 1.0)
                for h in range(HL):
                    nc.gpsimd.memset(vh[h][:], 0.0)
                for c in range(2):
                    nc.gpsimd.memset(ktz[c][0][64:128, :], 0.0)
                    nc.gpsimd.memset(ktz[c][1][0:64, :], 0.0)
                for sg in range(4):
                    for ec in range(2):
                        ps = pap.tile([128, 512], f32, tag="pa", name="pa")
                        for dc in range(8):
                            nc.tensor.matmul(
                                ps[:],
                                wvs[dc][:, 128 * ec:128 * (ec + 1)],
                                xts[dc][:, 512 * sg:512 * (sg + 1)],
                                start=(dc == 0), stop=(dc == 7))
                        vt = vtp.tile([128, 512], f32r, tag="vt", name="vt")
                        nc.vector.tensor_copy(vt[:], ps[:])
                        tr = trp.tile([128, 512], f32r, tag="tr", name="tr")
                        for i in range(4):
                            nc.tensor.transpose(
                                tr[:, 128 * i:128 * (i + 1)],
                                vt[:, 128 * i:128 * (i + 1)], id_sb[:])
                        for i in range(4):
                            sc = 4 * sg + i
                            for hh in range(2):
                                h = 2 * ec + hh
                                nc.scalar.copy(
                                    vh[h][:, 128 * sc + 64:128 * sc + 128],
                                    tr[:, 128 * i + 64 * hh:128 * i + 64 * hh + 64])
                for h in range(HL):
                    ones_col = vh[h][:].rearrange(
                        "p (s c) -> p s c", c=128)[:, :, 0:1]
                    nc.vector.tensor_copy(ones_col, ones16[:].rearrange("p (s c) -> p s c", c=1))

                # --- K then Q projections with RoPE (both single-matmul) ---
                # per 512-col chunk: project -> ACT-evacuate to SBUF ->
                # DMA block-swap (rotate-half partner) -> DVE cos/sin
                # multiplies + add.  The vector/DMA tail of chunk i overlaps
                # the PE chain of chunk i+1.
                for t_i, w_dram in enumerate([wk, wq]):
                    ws = [sa.tile([128, EL], dt.bfloat16, tag=f"w{dc}", name=f"w{dc}", bufs=3)
                          for dc in range(8)]
                    for dc in range(8):
                        nc.sync.dma_start(
                            ws[dc][:], w_dram[128 * dc:128 * (dc + 1), :])
                    for ec in range(2):
                        qa = qap.tile([128, S], f32, tag="qa", name="qa")
                        qas = qap.tile([128, S], f32, tag="qas", name="qas")
                        for sg in range(4):
                            sl = slice(512 * sg, 512 * (sg + 1))
                            ps = pap.tile([128, 512], f32, tag="pa", name="pa")
                            for dc in range(8):
                                nc.tensor.matmul(
                                    ps[:],
                                    ws[dc][:, 128 * ec:128 * (ec + 1)],
                                    xts[dc][:, 512 * sg:512 * (sg + 1)],
                                    start=(dc == 0), stop=(dc == 7))
                            nc.scalar.copy(qa[:, sl], ps[:])
                            # rotate-half partner: swap 32-partition blocks
                            for blk in range(2):
                                b0 = 64 * blk
                                nc.sync.dma_start(
                                    qas[b0:b0 + 32, sl], qa[b0 + 32:b0 + 64, sl])
                                nc.sync.dma_start(
                                    qas[b0 + 32:b0 + 64, sl], qa[b0:b0 + 32, sl])
                            qc = rop.tile([128, 512], f32, tag="qc", name="qc")
                            qs = rop.tile([128, 512], f32, tag="qs", name="qs")
                            nc.vector.tensor_mul(qc[:], qa[:, sl], cos_sb[:, sl])
                            nc.vector.tensor_mul(qs[:], qas[:, sl], sin_sb[:, sl])
                            if t_i == 0:
                                # K: write each head-half into its zero-padded
                                # stationary tile (other half stays zero) so
                                # score matmuls run with full K=128 activity
                                nc.vector.tensor_add(
                                    ktz[ec][0][0:64, sl], qc[0:64, :], qs[0:64, :])
                                nc.vector.tensor_add(
                                    ktz[ec][1][64:128, sl], qc[64:128, :], qs[64:128, :])
                            else:
                                nc.vector.tensor_add(qt[ec][:, sl], qc[:], qs[:])

            # ======== stage B: causal attention + output projection ========
            with tc.tile_pool(name="pb", bufs=1) as pb, \
                 tc.tile_pool(name="ptp", bufs=8) as ptp, \
                 tc.tile_pool(name="nrm", bufs=4) as nrmp, \
                 tc.tile_pool(name="ysb", bufs=2) as ysbp, \
                 tc.tile_pool(name="ps_s", bufs=2, space="PSUM") as ps_s, \
                 tc.tile_pool(name="ps_pv", bufs=2, space="PSUM") as ps_pv, \
                 tc.tile_pool(name="ps_y", bufs=1, space="PSUM") as ps_y, \
                 tc.tile_pool(name="ps_bc", bufs=1, space="PSUM") as ps_bc:

                aot = [pb.tile([128, S], dt.bfloat16, tag=f"aot{c}", name=f"aot{c}") for c in range(2)]
                wo_sb = [pb.tile([128, D], dt.bfloat16, tag=f"wo{c}", name=f"wo{c}") for c in range(2)]
                ones_sb = pb.tile([1, 64], f32r, tag="ones", name="ones")
                onesf = pb.tile([1, 64], f32, tag="onesf", name="onesf")
                nc.vector.memset(onesf[:], 1.0)
                nc.vector.tensor_copy(ones_sb[:], onesf[:])
                for c in range(2):
                    for half in range(2):
                        sl = slice(512 * half, 512 * (half + 1))
                        nc.scalar.dma_start(wo_sb[c][:, sl],
                                            wo[128 * c:128 * (c + 1), sl])

                def out_proj_chunk(sc):
                    ysb = ysbp.tile([128, D], dt.float16, tag="ysb", name="ysb")
                    for eg in range(2):
                        yp = ps_y.tile([128, 512], f32, tag="yp", name="yp")
                        for c2 in range(2):
                            nc.tensor.matmul(
                                yp[:],
                                aot[c2][:, 128 * sc:128 * (sc + 1)],
                                wo_sb[c2][:, 512 * eg:512 * (eg + 1)],
                                start=(c2 == 0), stop=(c2 == 1))
                        nc.vector.tensor_copy(
                            ysb[:, 512 * eg:512 * (eg + 1)], yp[:])
                    for half in range(2):
                        sl = slice(512 * half, 512 * (half + 1))
                        nc.sync.dma_start(
                            y[128 * sc:128 * (sc + 1), sl], ysb[:, sl])

                pending = []   # deferred normalize closures

                def emit_pending_one():
                    if pending:
                        pending.pop(0)()

                SKEW = 3
                for qg in range(4):
                    n_kc = 4 * qg + 4
                    # interleave deferred work (prev normalize + prev
                    # out-proj) through this q-group's attention stream
                    heat_at = {}
                    norm_at = {}
                    if qg >= 1:
                        k1 = max(1, n_kc // 4)
                        k2 = max(2, n_kc // 2)
                        k3 = max(3, (3 * n_kc) // 4)
                        norm_at = {(0, k1): 4}    # all 4 normalizes, early in hp0
                        heat_at = {(0, k2): 4 * (qg - 1) + 0,
                                   (0, k3): 4 * (qg - 1) + 1,
                                   (1, k1): 4 * (qg - 1) + 2,
                                   (1, k2): 4 * (qg - 1) + 3}
                    if qg == 3:
                        # flush this qg's hp0 normalizes during hp1 so only
                        # hp1's pair remains for the tail
                        norm_at[(1, 10)] = 2
                    for hp in range(2):           # head pairs (0,1), (2,3)
                        ppv = {}
                        for hh in range(2):
                            h = 2 * hp + hh
                            ppv[h] = ps_pv.tile([128, 512], f32, tag="ppv", name="ppv")
                        # software-pipelined: PV lags scores by SKEW so the
                        # exp+mask chain never head-of-line blocks the PE
                        ptq = {}
                        for kc in range(n_kc + SKEW):
                            for _ in range(norm_at.get((hp, kc), 0)):
                                emit_pending_one()
                            if (hp, kc) in heat_at:
                                out_proj_chunk(heat_at[(hp, kc)])
                            if kc < n_kc:
                                # diagonal tiles only need q >= k: narrow to
                                # the valid q-slice (saves PE+ACT on the
                                # mostly-masked tiles)
                                r = kc - 4 * qg
                                q0 = 128 * r if r > 0 else 0
                                qsl = slice(512 * qg + q0, 512 * (qg + 1))
                                ps2 = ps_s.tile([128, 1024], f32, tag="ps", name="ps")
                                for hh in range(2):
                                    nc.tensor.matmul(
                                        ps2[:, 512 * hh + q0:512 * (hh + 1)],
                                        ktz[hp][hh][:, 128 * kc:128 * (kc + 1)],
                                        qt[hp][:, qsl],
                                        start=True, stop=True)
                                pt = ptp.tile([128, 1024], dt.bfloat16, tag="pt", name="pt")
                                # one Exp over both heads' tiles (3D AP view)
                                psv = ps2[:].rearrange("p (h q) -> p h q", h=2)[:, :, q0:512]
                                ptv = pt[:].rearrange("p (h q) -> p h q", h=2)[:, :, q0:512]
                                nc.scalar.activation(
                                    ptv, psv,
                                    mybir.ActivationFunctionType.Exp,
                                    scale=0.125)
                                if r >= 0:
                                    for hh in range(2):
                                        nc.gpsimd.affine_select(
                                            pt[:, 512 * hh + q0:512 * (hh + 1)],
                                            pt[:, 512 * hh + q0:512 * (hh + 1)],
                                            pattern=[[1, 512 - q0]],
                                            compare_op=AluOpType.is_ge, fill=0.0,
                                            base=512 * qg + q0 - 128 * kc,
                                            channel_multiplier=-1)
                                ptq[kc] = (pt, q0)
                            kcp = kc - SKEW
                            if kcp >= 0:
                                ptv2, q0v = ptq.pop(kcp)
                                for hh in range(2):
                                    h = 2 * hp + hh
                                    nc.tensor.matmul(
                                        ppv[h][:, q0v:512],
                                        vh[h][:, 128 * kcp:128 * kcp + 128],
                                        ptv2[:, 512 * hh + q0v:512 * (hh + 1)],
                                        start=(kcp == 0), stop=(kcp == n_kc - 1))
                        # evacuate ppv fast: BOTH attn-out+denom copies first
                        # (they gate PSUM reuse for the next head pair), then
                        # the cheap approximate reciprocals
                        daos = []
                        for hh in range(2):
                            h = 2 * hp + hh
                            dao = nrmp.tile([128, 512], f32, tag="dao", name="dao")
                            nc.vector.tensor_copy(dao[:], ppv[h][:])
                            daos.append(dao)
                        for hh in range(2):
                            dao = daos[hh]
                            rec = nrmp.tile([1, 512], f32, tag="rec", name="rec")
                            nc.vector.reciprocal_approx_fast(
                                rec[0:1, :], dao[0:1, :])
                            recr = nrmp.tile([1, 512], f32r, tag="recr", name="recr")
                            nc.vector.tensor_copy(recr[:], rec[:])

                            def mk_norm(qg=qg, c2=hp, off=64 * hh, recr=recr, dao=dao):
                                def emit():
                                    # PE-broadcast 1/denom across the 64 head
                                    # dims, then normalize into aot
                                    bc = ps_bc.tile([64, 512], f32, tag="bc", name="bc")
                                    nc.tensor.matmul(bc[:], ones_sb[:],
                                                     recr[:],
                                                     start=True, stop=True)
                                    nc.vector.tensor_mul(
                                        aot[c2][off:off + 64, 512 * qg:512 * (qg + 1)],
                                        dao[64:128, :], bc[:])
                                return emit
                            pending.append(mk_norm())
                # tail: the two remaining hp1 normalizes, then the last four
                # out-projection chunks
                while pending:
                    emit_pending_one()
                for i in range(4):
                    out_proj_chunk(12 + i)

    nc.compile()
    return nc


def _prep_inputs(x, token_positions, Wq, Wk, Wv, Wo):
    # even/odd interleave permutation within each head (for rotate-half RoPE)
    perm = np.concatenate([np.arange(0, DK, 2), np.arange(1, DK, 2)])

    pos = np.asarray(token_positions).astype(np.float32)
    angles = THETA ** (-np.arange(32, dtype=np.float32) / 32.0)
    ang = pos[:, None] * angles[None, :]          # [S, 32]
    cos32 = np.cos(ang).T.astype(np.float32)      # [32, S]
    sin32 = np.sin(ang).T.astype(np.float32)
    cos128 = np.concatenate([cos32, cos32, cos32, cos32], axis=0)
    sin128 = np.concatenate([-sin32, sin32, -sin32, sin32], axis=0)
    cos128 = np.ascontiguousarray(cos128)
    sin128 = np.ascontiguousarray(sin128)

    identity = _round_fp32r(np.eye(128, dtype=np.float32))

    Wq = np.asarray(Wq, dtype=np.float32)
    Wk = np.asarray(Wk, dtype=np.float32)
    Wv = np.asarray(Wv, dtype=np.float32)
    Wo = np.asarray(Wo, dtype=np.float32)
    x = np.asarray(x, dtype=np.float32)

    in_maps = []
    for c in range(N_CORES):
        b = c // 4
        h0 = (c % 4) * HL
        esl = slice(h0 * DK, (h0 + HL) * DK)
        wq_h = Wq[esl].reshape(HL, DK, D)[:, perm].reshape(EL, D)
        wk_h = Wk[esl].reshape(HL, DK, D)[:, perm].reshape(EL, D)
        wv_h = Wv[esl]
        bf = lambda a: np.ascontiguousarray(a, dtype=np.float32).astype(ml_dtypes.bfloat16)
        in_maps.append({
            "xT": bf(x[b].T),
            "wq": bf(wq_h.T),
            "wk": bf(wk_h.T),
            "wv": bf(wv_h.T),
            "wo": bf(Wo[:, esl].T),
            "cosT": cos128,
            "sinT": sin128,
            "ident": identity,
        })
    return in_maps


def kernel(x, token_positions, Wq, Wk, Wv, Wo, _trace=False):
    from concourse.bass_utils import run_bass_kernel_spmd

    global _compiled
    if _compiled is None:
        _compiled = _build()
    in_maps = _prep_inputs(x, token_positions, Wq, Wk, Wv, Wo)
    res = run_bass_kernel_spmd(_compiled, in_maps, list(range(N_CORES)),
                               trace=_trace)
    parts = [res.results[c]["y"].astype(np.float64) for c in range(N_CORES)]
    out = np.empty((2, S, D), dtype=np.float32)
    out[0] = (parts[0] + parts[1] + parts[2] + parts[3]).astype(np.float32)
    out[1] = (parts[4] + parts[5] + parts[6] + parts[7]).astype(np.float32)
    if _trace:
        return out, res
    return out


# revision 10
# speedup vs baseline: 1.4185x; 1.0405x over previous
"""Trainium2 Bass kernel: multi-head self-attention with RoPE, causal mask.

Reference semantics (B=2, S=2048, D=1024, H=16, DK=64):
    q = rope(x @ Wq.T), k = rope(x @ Wk.T), v = x @ Wv.T   (per-head views)
    out = softmax(causal(q k^T / 8)) v ;  y = out @ Wo.T

Sharding over 8 cores: 2-way batch x 4-way heads (4 heads/core).
Each core computes a partial y [S, D] (its heads' contribution); host sums
the 4 partials per batch.

On-device layout strategy (per core):
  - host passes xT = x[b].T  [1024, 2048] so the d-contraction is on partitions
  - Q/K projections produce QT/KT [e, s]; head dims are even/odd-interleave
    permuted on the host so RoPE becomes a 32-partition block-swap + two
    multiplies (cos/sin tables with signs baked in)
  - Q and K are each projected ONCE; the rotate-half partner comes from a
    per-512-chunk SBUF-to-SBUF DMA block swap (issued on the idle gpsimd
    queue), with the rope multiplies chunked so they overlap the remaining
    projection matmuls on the PE
  - PSUM evacuations of the Q/K projections ride the ACT (scalar) engine,
    which is otherwise idle in stage A
  - scores are computed TRANSPOSED (k on partitions, q on free) so softmax'd
    probs feed the PV matmul directly as the moving operand
  - the two heads of a pair write their score tiles into ONE two-bank PSUM
    tile, so a single Exp activation instruction covers both (halves the
    ACT-engine instruction count, which binds stage B)
  - no max-subtraction in softmax (scores ~ N(0,1), exp is safe); the
    denominator comes from a ones-column appended to V; normalization is a
    fast approximate reciprocal + gpsimd partition-broadcast + multiply
  - all matmul operands are fp32r/bf16 (1 PE cycle/row)
"""

import sys

sys.path.insert(0, "/opt/trn_rl_repo")

import numpy as np
import ml_dtypes


S = 2048
D = 1024
NH = 16
DK = 64
HL = 4          # heads per core
EL = HL * DK    # 256 local e-dims
N_CORES = 8
THETA = 10000.0

_compiled = None


def _round_fp32r(x):
    # fp32r matmul operands must be pre-rounded to 11 mantissa bits (RNE)
    xi = np.ascontiguousarray(x, dtype=np.float32).view(np.uint32).astype(np.uint64)
    bias = ((xi >> 12) & 1) + (1 << 11) - 1
    return ((xi + bias) >> 12 << 12).astype(np.uint32).view(np.float32)


def _build():
    import concourse.bacc as bacc
    import concourse.tile as tile
    from concourse import mybir
    from concourse.alu_op_type import AluOpType

    dt = mybir.dt
    f32, f32r = dt.float32, dt.float32r

    nc = bacc.Bacc("TRN2", target_bir_lowering=False, debug=False,
                   num_devices=N_CORES)

    xT = nc.dram_tensor("xT", [D, S], dt.float16, kind="ExternalInput").ap()
    wq = nc.dram_tensor("wq", [D, EL], dt.float16, kind="ExternalInput").ap()
    wk = nc.dram_tensor("wk", [D, EL], dt.float16, kind="ExternalInput").ap()
    wv = nc.dram_tensor("wv", [D, EL], dt.float16, kind="ExternalInput").ap()
    wo = nc.dram_tensor("wo", [EL, D], dt.float16, kind="ExternalInput").ap()
    cosT = nc.dram_tensor("cosT", [128, S], dt.float16, kind="ExternalInput").ap()
    sinT = nc.dram_tensor("sinT", [128, S], dt.float16, kind="ExternalInput").ap()
    ident = nc.dram_tensor("ident", [128, 128], f32r, kind="ExternalInput").ap()
    y = nc.dram_tensor("y", [S, D], dt.float16, kind="ExternalOutput").ap()

    with tile.TileContext(nc) as tc:
        with tc.tile_pool(name="persist", bufs=1) as pp:
            # persistent SBUF tiles
            qt = [pp.tile([128, S], dt.float16, tag=f"qt{c}", name=f"qt{c}") for c in range(2)]
            ktz = [[pp.tile([128, S], dt.float16, tag=f"ktz{c}{par}", name=f"ktz{c}{par}")
                   for par in range(2)] for c in range(2)]
            vh = [pp.tile([128, 16 * 128], dt.float16, tag=f"v{h}", name=f"v{h}") for h in range(HL)]
            cos_sb = pp.tile([128, S], dt.float16, tag="cos", name="cos")
            sin_sb = pp.tile([128, S], dt.float16, tag="sin", name="sin")
            id_sb = pp.tile([128, 128], f32r, tag="ident", name="ident")

            # ======== stage A: projections + RoPE + V transpose ========
            # order: V first, then K, then Q — so the attention stream can
            # begin as soon as K and Q are rotated, keeping the PE dense
            # across the stage transition (HAM stays warm)
            with tc.tile_pool(name="stagea", bufs=1) as sa, \
                 tc.tile_pool(name="qap", bufs=2) as qap, \
                 tc.tile_pool(name="rop", bufs=3) as rop, \
                 tc.tile_pool(name="vtp", bufs=2) as vtp, \
                 tc.tile_pool(name="pa", bufs=4, space="PSUM") as pap, \
                 tc.tile_pool(name="tr", bufs=2, space="PSUM") as trp:

                xts = [sa.tile([128, S], dt.float16, tag=f"xt{dc}", name=f"xt{dc}") for dc in range(8)]
                def load_xt_slices(sg, split=1):
                    for dc in range(8):
                        eng = nc.sync if dc % 2 == 0 else nc.scalar
                        p = 128 // split
                        for j in range(split):
                            eng.dma_start(
                                xts[dc][p * j:p * (j + 1), 512 * sg:512 * (sg + 1)],
                                xT[128 * dc + p * j:128 * dc + p * (j + 1),
                                   512 * sg:512 * (sg + 1)])

                # warm up the PE clock-gate while input DMAs land: dummy
                # matmuls on the (early, tiny) identity tile keep HAM at 8/8
                # so the first real matmuls run at full clock
                nc.scalar.dma_start(id_sb[:], ident)
                idb = id_sb[:].bitcast(dt.bfloat16)   # values irrelevant
                wp = trp.tile([128, 512], f32, tag="warm", name="warm")
                for _ in range(50):
                    nc.tensor.matmul(wp[:, 0:256], idb[:, 0:128], idb[:],
                                     start=True, stop=True)

                # --- V projection (VT then PE-transpose into [s, dv]) ---
                load_xt_slices(0, split=2)
                wvs = [sa.tile([128, EL], dt.float16, tag=f"w{dc}", name=f"w{dc}", bufs=3)
                       for dc in range(8)]
                for dc in range(8):
                    nc.sync.dma_start(wvs[dc][:], wv[128 * dc:128 * (dc + 1), :])
                load_xt_slices(1)
                load_xt_slices(2)
                load_xt_slices(3)
                for half in range(2):
                    sl = slice(1024 * half, 1024 * (half + 1))
                    nc.scalar.dma_start(cos_sb[:, sl], cosT[:, sl])
                    nc.scalar.dma_start(sin_sb[:, sl], sinT[:, sl])

                ones16 = sa.tile([128, 16], f32, tag="ones16", name="ones16")
                nc.vector.memset(ones16[:], 1.0)
                for c in range(2):
                    nc.gpsimd.memset(ktz[c][0][64:128, :], 0.0)
                    nc.gpsimd.memset(ktz[c][1][0:64, :], 0.0)
                for sg in range(4):
                    for ec in range(2):
                        ps = pap.tile([128, 512], f32, tag="pa", name="pa")
                        for dc in range(8):
                            nc.tensor.matmul(
                                ps[:],
                                wvs[dc][:, 128 * ec:128 * (ec + 1)],
                                xts[dc][:, 512 * sg:512 * (sg + 1)],
                                start=(dc == 0), stop=(dc == 7))
                        vt = vtp.tile([128, 512], f32r, tag="vt", name="vt")
                        nc.vector.tensor_copy(vt[:], ps[:])
                        tr = trp.tile([128, 512], f32r, tag="tr", name="tr")
                        for i in range(4):
                            nc.tensor.transpose(
                                tr[:, 128 * i:128 * (i + 1)],
                                vt[:, 128 * i:128 * (i + 1)], id_sb[:])
                        for i in range(4):
                            sc = 4 * sg + i
                            for hh in range(2):
                                h = 2 * ec + hh
                                nc.scalar.copy(
                                    vh[h][:, 128 * sc + 64:128 * sc + 128],
                                    tr[:, 128 * i + 64 * hh:128 * i + 64 * hh + 64])
                for h in range(HL):
                    ones_col = vh[h][:].rearrange(
                        "p (s c) -> p s c", c=128)[:, :, 0:1]
                    nc.vector.tensor_copy(ones_col, ones16[:].rearrange("p (s c) -> p s c", c=1))

                # --- K then Q projections with RoPE (both single-matmul) ---
                # per 512-col chunk: project -> ACT-evacuate to SBUF ->
                # DMA block-swap (rotate-half partner) -> DVE cos/sin
                # multiplies + add.  The vector/DMA tail of chunk i overlaps
                # the PE chain of chunk i+1.
                for t_i, w_dram in enumerate([wk, wq]):
                    ws = [sa.tile([128, EL], dt.float16, tag=f"w{dc}", name=f"w{dc}", bufs=3)
                          for dc in range(8)]
                    for dc in range(8):
                        nc.sync.dma_start(
                            ws[dc][:], w_dram[128 * dc:128 * (dc + 1), :])
                    for ec in range(2):
                        qa = qap.tile([128, S], dt.float16, tag="qa", name="qa")
                        qas = qap.tile([128, S], dt.float16, tag="qas", name="qas")
                        for sg in range(4):
                            sl = slice(512 * sg, 512 * (sg + 1))
                            ps = pap.tile([128, 512], f32, tag="pa", name="pa")
                            for dc in range(8):
                                nc.tensor.matmul(
                                    ps[:],
                                    ws[dc][:, 128 * ec:128 * (ec + 1)],
                                    xts[dc][:, 512 * sg:512 * (sg + 1)],
                                    start=(dc == 0), stop=(dc == 7))
                            nc.scalar.copy(qa[:, sl], ps[:])
                            # rotate-half partner: swap 32-partition blocks
                            for blk in range(2):
                                b0 = 64 * blk
                                nc.sync.dma_start(
                                    qas[b0:b0 + 32, sl], qa[b0 + 32:b0 + 64, sl])
                                nc.sync.dma_start(
                                    qas[b0 + 32:b0 + 64, sl], qa[b0:b0 + 32, sl])
                            qc = rop.tile([128, 512], dt.float16, tag="qc", name="qc")
                            qs = rop.tile([128, 512], dt.float16, tag="qs", name="qs")
                            nc.vector.tensor_mul(qc[:], qa[:, sl], cos_sb[:, sl])
                            nc.gpsimd.tensor_mul(qs[:], qas[:, sl], sin_sb[:, sl])
                            if t_i == 0:
                                # K: write each head-half into its zero-padded
                                # stationary tile (other half stays zero) so
                                # score matmuls run with full K=128 activity
                                nc.vector.tensor_add(
                                    ktz[ec][0][0:64, sl], qc[0:64, :], qs[0:64, :])
                                nc.vector.tensor_add(
                                    ktz[ec][1][64:128, sl], qc[64:128, :], qs[64:128, :])
                            else:
                                nc.vector.tensor_add(qt[ec][:, sl], qc[:], qs[:])

            # ======== stage B: causal attention + output projection ========
            with tc.tile_pool(name="pb", bufs=1) as pb, \
                 tc.tile_pool(name="ptp", bufs=8) as ptp, \
                 tc.tile_pool(name="nrm", bufs=4) as nrmp, \
                 tc.tile_pool(name="ysb", bufs=2) as ysbp, \
                 tc.tile_pool(name="ps_s", bufs=2, space="PSUM") as ps_s, \
                 tc.tile_pool(name="ps_pv", bufs=2, space="PSUM") as ps_pv, \
                 tc.tile_pool(name="ps_y", bufs=1, space="PSUM") as ps_y, \
                 tc.tile_pool(name="ps_bc", bufs=1, space="PSUM") as ps_bc:

                aot = [pb.tile([128, S], dt.float16, tag=f"aot{c}", name=f"aot{c}") for c in range(2)]
                wo_sb = [pb.tile([128, D], dt.float16, tag=f"wo{c}", name=f"wo{c}") for c in range(2)]
                ones_sb = pb.tile([1, 64], f32r, tag="ones", name="ones")
                onesf = pb.tile([1, 64], f32, tag="onesf", name="onesf")
                nc.vector.memset(onesf[:], 1.0)
                nc.vector.tensor_copy(ones_sb[:], onesf[:])
                for c in range(2):
                    for half in range(2):
                        sl = slice(512 * half, 512 * (half + 1))
                        nc.scalar.dma_start(wo_sb[c][:, sl],
                                            wo[128 * c:128 * (c + 1), sl])

                def out_proj_chunk(sc, split=False):
                    ysb = ysbp.tile([128, D], dt.float16, tag="ysb", name="ysb")
                    for eg in range(2):
                        yp = ps_y.tile([128, 512], f32, tag="yp", name="yp")
                        for c2 in range(2):
                            nc.tensor.matmul(
                                yp[:],
                                aot[c2][:, 128 * sc:128 * (sc + 1)],
                                wo_sb[c2][:, 512 * eg:512 * (eg + 1)],
                                start=(c2 == 0), stop=(c2 == 1))
                        nc.vector.tensor_copy(
                            ysb[:, 512 * eg:512 * (eg + 1)], yp[:])
                    for half in range(2):
                        sl = slice(512 * half, 512 * (half + 1))
                        if split:
                            nc.sync.dma_start(
                                y[128 * sc:128 * sc + 64, sl], ysb[0:64, sl])
                            nc.scalar.dma_start(
                                y[128 * sc + 64:128 * (sc + 1), sl], ysb[64:128, sl])
                        else:
                            nc.sync.dma_start(
                                y[128 * sc:128 * (sc + 1), sl], ysb[:, sl])

                pending = []   # deferred normalize closures

                def emit_pending_one():
                    if pending:
                        pending.pop(0)()

                SKEW = 3
                for qg in range(4):
                    n_kc = 4 * qg + 4
                    # interleave deferred work (prev normalize + prev
                    # out-proj) through this q-group's attention stream
                    heat_at = {}
                    norm_at = {}
                    if qg >= 1:
                        k1 = max(1, n_kc // 4)
                        k2 = max(2, n_kc // 2)
                        k3 = max(3, (3 * n_kc) // 4)
                        norm_at = {(0, k1): 4}    # all 4 normalizes, early in hp0
                        heat_at = {(0, k2): 4 * (qg - 1) + 0,
                                   (0, k3): 4 * (qg - 1) + 1,
                                   (1, k1): 4 * (qg - 1) + 2,
                                   (1, k2): 4 * (qg - 1) + 3}
                    if qg == 3:
                        # flush this qg's hp0 normalizes during hp1 so only
                        # hp1's pair remains for the tail
                        norm_at[(1, 10)] = 2
                    for hp in range(2):           # head pairs (0,1), (2,3)
                        ppv = {}
                        for hh in range(2):
                            h = 2 * hp + hh
                            ppv[h] = ps_pv.tile([128, 512], f32, tag="ppv", name="ppv")
                        # software-pipelined: PV lags scores by SKEW so the
                        # exp+mask chain never head-of-line blocks the PE
                        ptq = {}
                        for kc in range(n_kc + SKEW):
                            for _ in range(norm_at.get((hp, kc), 0)):
                                emit_pending_one()
                            if (hp, kc) in heat_at:
                                out_proj_chunk(heat_at[(hp, kc)])
                            if kc < n_kc:
                                # diagonal tiles only need q >= k: narrow to
                                # the valid q-slice (saves PE+ACT on the
                                # mostly-masked tiles)
                                r = kc - 4 * qg
                                q0 = 128 * r if r > 0 else 0
                                qsl = slice(512 * qg + q0, 512 * (qg + 1))
                                ps2 = ps_s.tile([128, 1024], f32, tag="ps", name="ps")
                                for hh in range(2):
                                    nc.tensor.matmul(
                                        ps2[:, 512 * hh + q0:512 * (hh + 1)],
                                        ktz[hp][hh][:, 128 * kc:128 * (kc + 1)],
                                        qt[hp][:, qsl],
                                        start=True, stop=True)
                                pt = ptp.tile([128, 1024], dt.float16, tag="pt", name="pt")
                                # one Exp over both heads' tiles (3D AP view)
                                psv = ps2[:].rearrange("p (h q) -> p h q", h=2)[:, :, q0:512]
                                ptv = pt[:].rearrange("p (h q) -> p h q", h=2)[:, :, q0:512]
                                nc.scalar.activation(
                                    ptv, psv,
                                    mybir.ActivationFunctionType.Exp,
                                    scale=0.125)
                                if r >= 0:
                                    for hh in range(2):
                                        nc.gpsimd.affine_select(
                                            pt[:, 512 * hh + q0:512 * (hh + 1)],
                                            pt[:, 512 * hh + q0:512 * (hh + 1)],
                                            pattern=[[1, 512 - q0]],
                                            compare_op=AluOpType.is_ge, fill=0.0,
                                            base=512 * qg + q0 - 128 * kc,
                                            channel_multiplier=-1)
                                ptq[kc] = (pt, q0)
                            kcp = kc - SKEW
                            if kcp >= 0:
                                ptv2, q0v = ptq.pop(kcp)
                                for hh in range(2):
                                    h = 2 * hp + hh
                                    nc.tensor.matmul(
                                        ppv[h][:, q0v:512],
                                        vh[h][:, 128 * kcp:128 * kcp + 128],
                                        ptv2[:, 512 * hh + q0v:512 * (hh + 1)],
                                        start=(kcp == 0), stop=(kcp == n_kc - 1))
                        # evacuate ppv fast: BOTH attn-out+denom copies first
                        # (they gate PSUM reuse for the next head pair), then
                        # the cheap approximate reciprocals
                        daos = []
                        for hh in range(2):
                            h = 2 * hp + hh
                            dao = nrmp.tile([128, 512], f32, tag="dao", name="dao")
                            nc.vector.tensor_copy(dao[:], ppv[h][:])
                            daos.append(dao)
                        for hh in range(2):
                            dao = daos[hh]
                            rec = nrmp.tile([1, 512], f32, tag="rec", name="rec")
                            nc.vector.reciprocal_approx_fast(
                                rec[0:1, :], dao[0:1, :])
                            recr = nrmp.tile([1, 512], f32r, tag="recr", name="recr")
                            nc.vector.tensor_copy(recr[:], rec[:])

                            def mk_norm(qg=qg, c2=hp, off=64 * hh, recr=recr, dao=dao):
                                def emit():
                                    # PE-broadcast 1/denom across the 64 head
                                    # dims, then normalize into aot
                                    bc = ps_bc.tile([64, 512], f32, tag="bc", name="bc")
                                    nc.tensor.matmul(bc[:], ones_sb[:],
                                                     recr[:],
                                                     start=True, stop=True)
                                    nc.vector.tensor_mul(
                                        aot[c2][off:off + 64, 512 * qg:512 * (qg + 1)],
                                        dao[64:128, :], bc[:])
                                return emit
                            pending.append(mk_norm())
                # tail: the two remaining hp1 normalizes, then the last four
                # out-projection chunks
                while pending:
                    emit_pending_one()
                for i in range(4):
                    out_proj_chunk(12 + i, split=True)

    nc.compile()
    return nc


def _prep_inputs(x, token_positions, Wq, Wk, Wv, Wo):
    # even/odd interleave permutation within each head (for rotate-half RoPE)
    perm = np.concatenate([np.arange(0, DK, 2), np.arange(1, DK, 2)])

    pos = np.asarray(token_positions).astype(np.float32)
    angles = THETA ** (-np.arange(32, dtype=np.float32) / 32.0)
    ang = pos[:, None] * angles[None, :]          # [S, 32]
    cos32 = np.cos(ang).T.astype(np.float32)      # [32, S]
    sin32 = np.sin(ang).T.astype(np.float32)
    cos128 = np.concatenate([cos32, cos32, cos32, cos32], axis=0)
    sin128 = np.concatenate([-sin32, sin32, -sin32, sin32], axis=0)
    cos128 = np.ascontiguousarray(cos128).astype(np.float16)
    sin128 = np.ascontiguousarray(sin128).astype(np.float16)

    identity = _round_fp32r(np.eye(128, dtype=np.float32))

    Wq = np.asarray(Wq, dtype=np.float32)
    Wk = np.asarray(Wk, dtype=np.float32)
    Wv = np.asarray(Wv, dtype=np.float32)
    Wo = np.asarray(Wo, dtype=np.float32)
    x = np.asarray(x, dtype=np.float32)

    in_maps = []
    for c in range(N_CORES):
        b = c // 4
        h0 = (c % 4) * HL
        esl = slice(h0 * DK, (h0 + HL) * DK)
        wq_h = Wq[esl].reshape(HL, DK, D)[:, perm].reshape(EL, D)
        wk_h = Wk[esl].reshape(HL, DK, D)[:, perm].reshape(EL, D)
        wv_h = Wv[esl]
        bf = lambda a: np.ascontiguousarray(a, dtype=np.float32).astype(np.float16)
        in_maps.append({
            "xT": bf(x[b].T),
            "wq": bf(wq_h.T),
            "wk": bf(wk_h.T),
            "wv": bf(wv_h.T),
            "wo": bf(Wo[:, esl].T),
            "cosT": cos128,
            "sinT": sin128,
            "ident": identity,
        })
    return in_maps


def kernel(x, token_positions, Wq, Wk, Wv, Wo, _trace=False):
    from concourse.bass_utils import run_bass_kernel_spmd

    global _compiled
    if _compiled is None:
        _compiled = _build()
    in_maps = _prep_inputs(x, token_positions, Wq, Wk, Wv, Wo)
    res = run_bass_kernel_spmd(_compiled, in_maps, list(range(N_CORES)),
                               trace=_trace)
    parts = [res.results[c]["y"].astype(np.float64) for c in range(N_CORES)]
    out = np.empty((2, S, D), dtype=np.float32)
    out[0] = (parts[0] + parts[1] + parts[2] + parts[3]).astype(np.float32)
    out[1] = (parts[4] + parts[5] + parts[6] + parts[7]).astype(np.float32)
    if _trace:
        return out, res
    return out
